# revision 3
# baseline (speedup 1.0000x reference)
"""Trainium2 Bass kernel for the 3-view attention-fusion pooling module.

Computation (reference):
    t_k  = tanh(W @ x_k)                      (A=256, D=256), k = 1..3
    s_k  = h_n @ t_k                          (1, D)
    beta = softmax([s_1; s_2; s_3], axis=0)   (3, D)
    out  = beta[0]*x1 + beta[1]*x2 + beta[2]*x3   (N, D)

Sharding: rows (node dim N=100000) split evenly across 8 cores. W is fed
per-core as W[:, shard].T (host-side transpose) so the contraction dim is
the partition dim for the TensorE matmul. The (A, D) GEMM partials are
AllReduce-summed across cores; everything downstream of the reduction is
tiny and computed redundantly on every core.
"""

import sys

import numpy as np

for _p in ("/opt/trn_rl_repo", "/root/.axon_site/_ro/trn_rl_repo"):
    if _p not in sys.path:
        sys.path.append(_p)

import concourse.bacc as bacc
import concourse.tile as tile
from concourse import mybir
from concourse.bass_utils import run_bass_kernel_spmd

N_CORES = 8
N = 100000
D = 256          # feature dim
A = 256          # input_att
N_LOC = N // N_CORES   # 12500 rows per core
P = 125          # contraction-chunk rows (partition dim of matmul operands)
G = 10           # chunks per DMA batch
NB = N_LOC // (P * G)  # 10 batches
FW = G * D       # free width of a batched SBUF tile

FP32 = mybir.dt.float32
MM_DT = mybir.dt.float32r  # matmul compute dtype (same storage as fp32)


def build_bass():
    nc = bacc.Bacc("TRN2", target_bir_lowering=False, debug=False,
                   num_devices=N_CORES)

    x1 = nc.dram_tensor("x1", [N_LOC, D], FP32, kind="ExternalInput")
    x2 = nc.dram_tensor("x2", [N_LOC, D], FP32, kind="ExternalInput")
    x3 = nc.dram_tensor("x3", [N_LOC, D], FP32, kind="ExternalInput")
    wt = nc.dram_tensor("wt", [N_LOC, A], FP32, kind="ExternalInput")
    hnt = nc.dram_tensor("hnt", [A, 1], FP32, kind="ExternalInput")
    out = nc.dram_tensor("out", [N_LOC, D], FP32, kind="ExternalOutput")

    Tanh = mybir.ActivationFunctionType.Tanh
    Exp = mybir.ActivationFunctionType.Exp

    with tile.TileContext(nc) as tc:
        with (
            tc.tile_pool(name="px1", bufs=3) as px1,
            tc.tile_pool(name="px2", bufs=3) as px2,
            tc.tile_pool(name="px3", bufs=3) as px3,
            tc.tile_pool(name="pw", bufs=2) as pw,
            tc.tile_pool(name="pout", bufs=2) as pout,
            tc.tile_pool(name="ptmp", bufs=2) as ptmp,
            tc.tile_pool(name="small", bufs=1) as small,
            tc.tile_pool(name="pdram", bufs=1, space="DRAM") as pdram,
        ):
            # batched row views: batch b covers rows [b*1250, (b+1)*1250)
            x1r = x1.ap().rearrange("(b g p) d -> b p g d", p=P, g=G)
            x2r = x2.ap().rearrange("(b g p) d -> b p g d", p=P, g=G)
            x3r = x3.ap().rearrange("(b g p) d -> b p g d", p=P, g=G)
            wtr = wt.ap().rearrange("(b g p) a -> b p g a", p=P, g=G)
            outr = out.ap().rearrange("(b g p) d -> b p g d", p=P, g=G)
            xrs = (x1r, x2r, x3r)
            xpools = (px1, px2, px3)

            # h_n laid out [a_half(128 partitions), h(2)]
            hn_sb = small.tile([128, 2], FP32, tag="hn")
            nc.sync.dma_start(hn_sb[:, :],
                              hnt.ap().rearrange("(h a) o -> a (h o)", h=2))
            ones_sb = small.tile([1, 128], FP32, tag="ones")
            nc.vector.memset(ones_sb[:], 1.0)

            # ---------------- phase 1: u_k = W @ x_k (per-core partials) ---
            cc_in = small.tile([128, 6 * D], FP32, tag="cc_in")
            with tc.tile_pool(name="pacc", bufs=1, space="PSUM") as pacc:
                uacc = [[pacc.tile([128, D], FP32, name=f"u{v}{h}", tag=f"u{v}{h}")
                         for h in range(2)] for v in range(3)]
                for b in range(NB):
                    xts = []
                    for v in range(3):
                        t = xpools[v].tile([P, FW], MM_DT, name=f"x{v}", tag="x")
                        nc.sync.dma_start(
                            t[:].rearrange("p (g d) -> p g d", g=G),
                            xrs[v][b].bitcast(MM_DT))
                        xts.append(t)
                    wtile = pw.tile([P, FW], MM_DT, name="w", tag="w")
                    nc.sync.dma_start(
                        wtile[:].rearrange("p (g a) -> p g a", g=G),
                        wtr[b].bitcast(MM_DT))
                    for g in range(G):
                        first = (b == 0 and g == 0)
                        last = (b == NB - 1 and g == G - 1)
                        for h in range(2):
                            lhs = wtile[:, g * A + h * 128: g * A + h * 128 + 128]
                            for v in range(3):
                                nc.tensor.matmul(
                                    uacc[v][h][:],
                                    lhsT=lhs,
                                    rhs=xts[v][:, g * D:(g + 1) * D],
                                    start=first, stop=last)
                for v in range(3):
                    for h in range(2):
                        i = v * 2 + h
                        nc.vector.tensor_copy(cc_in[:, i * D:(i + 1) * D],
                                              uacc[v][h][:])

            # ---------------- all-reduce the GEMM partials ------------------
            ccin_d = pdram.tile([128, 6 * D], FP32, tag="ccin")
            ccout_d = pdram.tile([128, 6 * D], FP32, tag="ccout")
            nc.sync.dma_start(ccin_d[:], cc_in[:])
            nc.gpsimd.collective_compute(
                "AllReduce", mybir.AluOpType.add,
                replica_groups=[list(range(N_CORES))],
                ins=[ccin_d.opt()], outs=[ccout_d.opt()])
            t_red = small.tile([128, 6 * D], FP32, tag="t_red")
            nc.sync.dma_start(t_red[:], ccout_d[:])

            # ---------------- tanh, scores, softmax, beta broadcast ---------
            t_tanh = small.tile([128, 6 * D], FP32, tag="t_tanh")
            nc.scalar.activation(t_tanh[:], t_red[:], Tanh)

            evs = []
            Bsb = []
            with (
                tc.tile_pool(name="ps", bufs=1, space="PSUM") as ps,
                tc.tile_pool(name="pB", bufs=1, space="PSUM") as pB,
            ):
                for v in range(3):
                    s_ps = ps.tile([1, D], FP32, name=f"s{v}", tag=f"s{v}")
                    for h in range(2):
                        i = v * 2 + h
                        nc.tensor.matmul(
                            s_ps[:], lhsT=hn_sb[:, h:h + 1],
                            rhs=t_tanh[:, i * D:(i + 1) * D],
                            start=(h == 0), stop=(h == 1))
                    e_v = small.tile([1, D], FP32, name=f"e{v}", tag=f"e{v}")
                    nc.scalar.activation(e_v[:], s_ps[:], Exp)
                    evs.append(e_v)
                ssum = small.tile([1, D], FP32, tag="ssum")
                nc.vector.tensor_add(ssum[:], evs[0][:], evs[1][:])
                nc.vector.tensor_add(ssum[:], ssum[:], evs[2][:])
                rinv = small.tile([1, D], FP32, tag="rinv")
                nc.vector.reciprocal(rinv[:], ssum[:])
                for v in range(3):
                    b_v = small.tile([1, D], FP32, name=f"bt{v}", tag=f"bt{v}")
                    nc.vector.tensor_mul(b_v[:], evs[v][:], rinv[:])
                    B_ps = pB.tile([128, D], FP32, name=f"B{v}", tag=f"B{v}")
                    nc.tensor.matmul(B_ps[:], lhsT=ones_sb[:], rhs=b_v[:],
                                     start=True, stop=True)
                    B_v = small.tile([128, D], FP32, name=f"Bb{v}", tag=f"Bb{v}")
                    nc.vector.tensor_copy(B_v[:], B_ps[:])
                    Bsb.append(B_v)

            # ---------------- phase 2: out = sum_k beta_k * x_k -------------
            Bb = [Bsb[v][0:P, :].unsqueeze(1).broadcast_to([P, G, D])
                  for v in range(3)]
            for b in range(NB):
                xts = []
                for v in range(3):
                    t = xpools[v].tile([P, FW], FP32, name=f"x{v}", tag="x")
                    nc.sync.dma_start(
                        t[:].rearrange("p (g d) -> p g d", g=G), xrs[v][b])
                    xts.append(t)
                ot = pout.tile([P, FW], FP32, name="o", tag="o")
                tmp = ptmp.tile([P, FW], FP32, name="t", tag="t")
                o3 = ot[:].rearrange("p (g d) -> p g d", g=G)
                t3 = tmp[:].rearrange("p (g d) -> p g d", g=G)
                xs3 = [t[:].rearrange("p (g d) -> p g d", g=G) for t in xts]
                nc.vector.tensor_mul(o3, xs3[0], Bb[0])
                nc.vector.tensor_mul(t3, xs3[1], Bb[1])
                nc.vector.tensor_add(o3, o3, t3)
                nc.vector.tensor_mul(t3, xs3[2], Bb[2])
                nc.vector.tensor_add(o3, o3, t3)
                nc.sync.dma_start(outr[b], o3)

    nc.compile()
    return nc


_NC_CACHE = {}


def _get_nc():
    if "nc" not in _NC_CACHE:
        _NC_CACHE["nc"] = build_bass()
    return _NC_CACHE["nc"]


def kernel(x1, x2, x3, W, h_n):
    x1 = np.ascontiguousarray(x1, dtype=np.float32)
    x2 = np.ascontiguousarray(x2, dtype=np.float32)
    x3 = np.ascontiguousarray(x3, dtype=np.float32)
    W = np.ascontiguousarray(W, dtype=np.float32)
    h_n = np.ascontiguousarray(h_n, dtype=np.float32)

    hnt = np.ascontiguousarray(h_n.reshape(-1)[:, None])  # (A, 1)
    in_maps = []
    for c in range(N_CORES):
        sl = slice(c * N_LOC, (c + 1) * N_LOC)
        in_maps.append({
            "x1": x1[sl],
            "x2": x2[sl],
            "x3": x3[sl],
            "wt": np.ascontiguousarray(W[:, sl].T),
            "hnt": hnt,
        })

    nc = _get_nc()
    res = run_bass_kernel_spmd(nc, in_maps, core_ids=list(range(N_CORES)))
    return np.concatenate([res.results[c]["out"] for c in range(N_CORES)],
                          axis=0)
